# revision 1
# baseline (speedup 1.0000x reference)
"""Trainium2 Bass kernel for a 3-layer LSTM (B=64, T=256, F=64, H=1024)
+ tanh output projection, SPMD across 8 NeuronCores.

Strategy: shard the 4H=4096 gate dimension 8 ways (each core owns a
512-wide gate slice == a 128-wide h-slice per layer), keep the full batch
B=64 on every core. Per time step each core computes its gate slice
(h.T stationary [128,64] x weight moving [128,512] matmuls, PSUM
accumulated over K), does the LSTM elementwise on its slice, transposes
its h-slice to [128,64] via the PE, and AllGathers it so every core has
the full h.T for the next step. The 3 layers are software-pipelined:
slot s computes L1(t=s), L2(t=s-1), L3(t=s-2), so the AllGather latency
of each layer hides behind the other layers' matmuls. The output
projection runs every 8 slots from SBUF-resident gathered h3.
"""

import sys

sys.path.insert(0, "/opt/trn_rl_repo")

import numpy as np

B, T, F, H = 64, 256, 64, 1024
R = 8           # cores
G = 4 * H // R  # 512 gate slice per core
HS = H // R     # 128 h slice per core
YG = 8          # y-projection group size (slots)

_GATE_ORDER = (0, 1, 3, 2)  # i, f, o, g (PyTorch row blocks i,f,g,o)


def _slice_rows(w, r):
    """Rows of a [4H, *] weight for core r, in i|f|o|g block order."""
    return np.concatenate([w[g * H + HS * r: g * H + HS * (r + 1)] for g in _GATE_ORDER], 0)


def _k_tiles(wT):
    """[K, G] -> [128, K/128, G] SBUF layout (partition-major k-tiles)."""
    K = wT.shape[0]
    return np.ascontiguousarray(
        wT.reshape(K // 128, 128, wT.shape[1]).transpose(1, 0, 2)
    )


def _prep_core_inputs(r, X, weights):
    (w_ih1, w_hh1, b_ih1, b_hh1, w_ih2, w_hh2, b_ih2, b_hh2,
     w_ih3, w_hh3, b_ih3, b_hh3, w_out, b_out) = weights
    f32 = np.float32
    inp = {}
    # X: [B, T, F] -> [T, 128(pad F), B]
    Xt = np.zeros((T, 128, B), f32)
    Xt[:, :F, :] = X.transpose(1, 2, 0)
    inp["Xt"] = Xt
    # layer 1 input weight: [512, F].T -> [F, 512] -> pad K to 128
    w1 = _slice_rows(w_ih1, r).T  # [F, 512]
    W1x = np.zeros((128, G), f32)
    W1x[:F] = w1
    inp["W1x"] = W1x
    inp["Whh1"] = _k_tiles(_slice_rows(w_hh1, r).T.astype(f32))
    inp["Wih2"] = _k_tiles(_slice_rows(w_ih2, r).T.astype(f32))
    inp["Whh2"] = _k_tiles(_slice_rows(w_hh2, r).T.astype(f32))
    inp["Wih3"] = _k_tiles(_slice_rows(w_ih3, r).T.astype(f32))
    inp["Whh3"] = _k_tiles(_slice_rows(w_hh3, r).T.astype(f32))
    def brow(bi, bh):
        return _slice_rows((bi + bh).reshape(4 * H, 1), r)[:, 0]  # [512]

    inp["B1"] = np.broadcast_to(brow(b_ih1, b_hh1), (B, G)).astype(f32).copy()
    inp["B23"] = np.concatenate([
        np.broadcast_to(brow(b_ih2, b_hh2), (B, G)),
        np.broadcast_to(brow(b_ih3, b_hh3), (B, G)),
    ], 0).astype(f32).copy()
    # output projection: w_out [F, H] -> lhsT tiles [128, 8, F]
    inp["Wout"] = _k_tiles(np.ascontiguousarray(w_out.T).astype(f32))
    inp["bout"] = b_out.reshape(F, 1).astype(f32)
    return inp


def build_nc(t_steps=T):
    import concourse.bass as bass
    import concourse.mybir as mybir
    import concourse.tile as tile
    from concourse import bacc
    from concourse.masks import make_identity

    f32 = mybir.dt.float32
    AF = mybir.ActivationFunctionType
    NSLOT = t_steps + 2
    rg = [list(range(R))]

    nc = bacc.Bacc("TRN2", target_bir_lowering=False, debug=False, num_devices=R)

    p_Xt = nc.dram_tensor("Xt", [T, 128, B], f32, kind="ExternalInput")
    p_W1x = nc.dram_tensor("W1x", [128, G], f32, kind="ExternalInput")
    pw = {}
    for name in ("Whh1", "Wih2", "Whh2", "Wih3", "Whh3"):
        pw[name] = nc.dram_tensor(name, [128, 8, G], f32, kind="ExternalInput")
    p_B1 = nc.dram_tensor("B1", [B, G], f32, kind="ExternalInput")
    p_B23 = nc.dram_tensor("B23", [2 * B, G], f32, kind="ExternalInput")
    p_Wout = nc.dram_tensor("Wout", [128, 8, F], f32, kind="ExternalInput")
    p_bout = nc.dram_tensor("bout", [F, 1], f32, kind="ExternalInput")
    p_Y = nc.dram_tensor("Y", [F, t_steps * B], f32, kind="ExternalOutput")

    with tile.TileContext(nc) as tc:
        with (
            tc.tile_pool(name="wpool", bufs=1) as wpool,
            tc.tile_pool(name="state", bufs=1) as state,
            tc.tile_pool(name="xq", bufs=4) as xq,
            tc.tile_pool(name="hq", bufs=3) as hq,
            tc.tile_pool(name="h3g", bufs=2) as h3g,
            tc.tile_pool(name="sbt", bufs=2) as sbt,
            tc.tile_pool(name="gps", bufs=3, space="PSUM") as gps,
            tc.tile_pool(name="tps", bufs=2, space="PSUM") as tps,
            tc.tile_pool(name="yps", bufs=1, space="PSUM") as yps,
            tc.tile_pool(name="dms", bufs=4, space="DRAM") as dms,
        ):
            # ---- resident weights ----
            W1x = wpool.tile([128, G], f32, tag="W1x")
            nc.sync.dma_start(W1x[:], p_W1x[:])
            W = {}
            for name in ("Whh1", "Wih2", "Whh2", "Wih3", "Whh3"):
                W[name] = wpool.tile([128, 8, G], f32, tag=name, name=name + "_sb")
                nc.sync.dma_start(W[name][:], pw[name][:])
            B1sb = wpool.tile([B, G], f32, tag="B1", name="B1_sb")
            nc.sync.dma_start(B1sb[:], p_B1[:])
            B23sb = wpool.tile([2 * B, G], f32, tag="B23", name="B23_sb")
            nc.sync.dma_start(B23sb[:], p_B23[:])
            Bias = {1: B1sb, 2: B23sb[0:B], 3: B23sb[B:2 * B]}
            Wout = wpool.tile([128, 8, F], f32, tag="Wout")
            nc.sync.dma_start(Wout[:], p_Wout[:])
            bout = wpool.tile([F, 1], f32, tag="bout")
            nc.sync.dma_start(bout[:], p_bout[:])
            ident = wpool.tile([128, 128], f32, tag="ident")
            make_identity(nc, ident[:])

            # ---- persistent state ----
            c1t = state.tile([B, HS], f32, tag="c1", name="c1")
            nc.gpsimd.memset(c1t[:], 0.0)
            c23t = state.tile([2 * B, HS], f32, tag="c23", name="c23")
            nc.gpsimd.memset(c23t[:], 0.0)
            cst = {1: c1t, 2: c23t[0:B], 3: c23t[B:2 * B]}

            # gathered h.T tiles: H[layer][t]
            Hts = {1: {}, 2: {}, 3: {}}
            h3_group = {}  # group idx -> tile [128, YG, 8, B]

            def lstm_elementwise(key, gpsum, bias, c, P):
                """gates psum [P, G] -> h [P, HS] in SBUF (P=64 solo, 128 packed)."""
                gsb = sbt.tile([P, G], f32, tag=f"gsb{key}", name=f"gsb{key}")
                nc.vector.tensor_add(out=gsb[:], in0=gpsum[:], in1=bias)
                sio = sbt.tile([P, 3 * HS], f32, tag=f"sio{key}", name=f"sio{key}")
                nc.scalar.activation(sio[:], gsb[:, 0:3 * HS], AF.Sigmoid)
                tg = sbt.tile([P, HS], f32, tag=f"tg{key}", name=f"tg{key}")
                nc.scalar.activation(tg[:], gsb[:, 3 * HS:4 * HS], AF.Tanh)
                fc = sbt.tile([P, HS], f32, tag=f"fc{key}", name=f"fc{key}")
                nc.vector.tensor_mul(out=fc[:], in0=sio[:, HS:2 * HS], in1=c)
                ig = sbt.tile([P, HS], f32, tag=f"ig{key}", name=f"ig{key}")
                nc.vector.tensor_mul(out=ig[:], in0=sio[:, 0:HS], in1=tg[:])
                nc.vector.tensor_add(out=c, in0=fc[:], in1=ig[:])
                tc_ = sbt.tile([P, HS], f32, tag=f"tc{key}", name=f"tc{key}")
                nc.scalar.activation(tc_[:], c, AF.Tanh)
                h = sbt.tile([P, HS], f32, tag=f"h{key}", name=f"h{key}")
                nc.vector.tensor_mul(out=h[:], in0=sio[:, 2 * HS:3 * HS], in1=tc_[:])
                return h

            def stage_ag(key, h, P):
                """transpose h [P, HS] -> [HS, P], stage to DRAM, issue AG."""
                pt = tps.tile([HS, 2 * B], f32, tag="pt", name=f"pt{key}")[:, 0:P]
                nc.tensor.transpose(pt[:], h[:], ident[0:P, 0:P])
                hts = sbt.tile([HS, P], f32, tag=f"hts{key}", name=f"hts{key}")
                nc.vector.tensor_copy(out=hts[:], in_=pt[:])
                agin = dms.tile([HS, P], f32, tag=f"agin{key}", name=f"agin{key}")
                nc.gpsimd.dma_start(agin[:], hts[:])
                agout = dms.tile([R, HS, P], f32, tag=f"agout{key}", name=f"agout{key}")
                nc.gpsimd.collective_compute(
                    "AllGather", mybir.AluOpType.bypass,
                    replica_groups=rg, ins=[agin[:].opt()], outs=[agout[:].opt()],
                )
                return agout

            def fetch_ag(li, agout, t):
                if li == 3:
                    g, j = t // YG, t % YG
                    if j == 0:
                        h3_group[g] = h3g.tile([128, YG, 8, B], f32, tag="h3grp", name="h3grp")
                    Ht = h3_group[g][:, j]  # [128, 8, B]
                else:
                    Ht = hq.tile([128, 8, B], f32, tag=f"H{li}", name=f"H{li}")
                nc.gpsimd.dma_start(Ht[:], agout[:].rearrange("r p b -> p r b"))
                Hts[li][t] = Ht

            for s in range(NSLOT):
                staged = []
                t2, t3 = s - 1, s - 2
                l2_active = 0 <= t2 < t_steps
                l3_active = 0 <= t3 < t_steps
                packed = l2_active and l3_active

                # ---------- layer 1 matmuls: t = s ----------
                if s < t_steps:
                    xs = xq.tile([128, B], f32)
                    nc.sync.dma_start(xs[:], p_Xt[s])
                    g1 = gps.tile([2 * B, G], f32, tag="g", name="g1")[0:B]
                    nc.tensor.matmul(g1[:], xs[:], W1x[:], start=True, stop=(s == 0))
                    if s > 0:
                        hp = Hts[1][s - 1]
                        for k in range(8):
                            nc.tensor.matmul(g1[:], hp[:, k], W["Whh1"][:, k],
                                             start=False, stop=(k == 7))

                # ---------- layers 2+3 matmuls (packed when both active) ----
                if packed:
                    g23 = gps.tile([2 * B, G], f32, tag="g", name="g23")
                    gl2 = g23[0:B]
                    gl3 = g23[B:2 * B]
                    hp1 = Hts[1][t2]
                    hp2 = Hts[2][t3]
                    # interleave base-0 (L2) and base-64 (L3) matmuls so they
                    # run in different PE column groups concurrently
                    l2_mms = [(hp1[:, k], W["Wih2"][:, k]) for k in range(8)]
                    if t2 > 0:
                        hp2b = Hts[2][t2 - 1]
                        l2_mms += [(hp2b[:, k], W["Whh2"][:, k]) for k in range(8)]
                    l3_mms = [(hp2[:, k], W["Wih3"][:, k]) for k in range(8)]
                    if t3 > 0:
                        hp3 = Hts[3][t3 - 1]
                        l3_mms += [(hp3[:, k], W["Whh3"][:, k]) for k in range(8)]
                    n = max(len(l2_mms), len(l3_mms))
                    for i in range(n):
                        if i < len(l2_mms):
                            lhsT, rhs = l2_mms[i]
                            nc.tensor.matmul(gl2, lhsT, rhs, start=(i == 0),
                                             stop=(i == len(l2_mms) - 1),
                                             skip_group_check=True)
                        if i < len(l3_mms):
                            lhsT, rhs = l3_mms[i]
                            nc.tensor.matmul(gl3, lhsT, rhs, start=(i == 0),
                                             stop=(i == len(l3_mms) - 1),
                                             skip_group_check=True)
                elif l2_active:  # s == 1 (first L2 step) or tail
                    g2 = gps.tile([2 * B, G], f32, tag="g", name="g2")[0:B]
                    hp1 = Hts[1][t2]
                    for k in range(8):
                        nc.tensor.matmul(g2[:], hp1[:, k], W["Wih2"][:, k],
                                         start=(k == 0), stop=(t2 == 0 and k == 7))
                    if t2 > 0:
                        hp2b = Hts[2][t2 - 1]
                        for k in range(8):
                            nc.tensor.matmul(g2[:], hp2b[:, k], W["Whh2"][:, k],
                                             start=False, stop=(k == 7))
                elif l3_active:  # s == T+1 (last L3 step)
                    g3 = gps.tile([2 * B, G], f32, tag="g", name="g3")[0:B]
                    hp2 = Hts[2][t3]
                    for k in range(8):
                        nc.tensor.matmul(g3[:], hp2[:, k], W["Wih3"][:, k],
                                         start=(k == 0), stop=False)
                    hp3 = Hts[3][t3 - 1]
                    for k in range(8):
                        nc.tensor.matmul(g3[:], hp3[:, k], W["Whh3"][:, k],
                                         start=False, stop=(k == 7))

                # ---------- elementwise + exchange ----------
                if s < t_steps:
                    h1 = lstm_elementwise("1", g1, Bias[1][:], cst[1], B)
                    staged.append((1, stage_ag("1", h1, B), s))
                    if s - 2 >= 0:
                        Hts[1].pop(s - 2, None)
                if packed:
                    h23 = lstm_elementwise("23", g23, B23sb[:], c23t[:], 2 * B)
                    staged.append((23, stage_ag("23", h23, 2 * B), t2))
                    if t2 - 2 >= 0:
                        Hts[2].pop(t2 - 2, None)
                    if t3 - 2 >= 0:
                        Hts[3].pop(t3 - 2, None)
                elif l2_active:
                    h2 = lstm_elementwise("2", g2, Bias[2], cst[2], B)
                    staged.append((2, stage_ag("2", h2, B), t2))
                elif l3_active:
                    # last L3 step: c3 lives at partitions 64-127; copy down
                    c3tmp = sbt.tile([B, HS], f32, tag="c3tmp", name="c3tmp")
                    nc.sync.dma_start(c3tmp[:], cst[3])
                    h3 = lstm_elementwise("3", g3, Bias[3], c3tmp[:], B)
                    staged.append((3, stage_ag("3", h3, B), t3))

                # ---------- fetch AG results ----------
                def fetch_h1(agout, t):
                    Ht = hq.tile([128, 8, B], f32, tag="H1", name="H1")
                    nc.gpsimd.dma_start(Ht[:], agout[:].rearrange("r p b -> p r b"))
                    Hts[1][t] = Ht

                def fetch_h2(agout, t, col0):
                    Ht = hq.tile([128, 8, B], f32, tag="H2", name="H2")
                    nc.gpsimd.dma_start(
                        Ht[:], agout[:, :, col0:col0 + B].rearrange("r p b -> p r b"))
                    Hts[2][t] = Ht

                def fetch_h3(agout, t, col0):
                    g_, j_ = t // YG, t % YG
                    if j_ == 0:
                        h3_group[g_] = h3g.tile([128, YG, 8, B], f32,
                                                tag="h3grp", name="h3grp")
                    Ht = h3_group[g_][:, j_]
                    nc.gpsimd.dma_start(
                        Ht[:], agout[:, :, col0:col0 + B].rearrange("r p b -> p r b"))
                    Hts[3][t] = Ht

                for key, agout, t in staged:
                    if key == 1:
                        fetch_h1(agout, t)
                    elif key == 2:
                        fetch_h2(agout, t, 0)
                    elif key == 3:
                        fetch_h3(agout, t, 0)
                    else:  # packed 23
                        fetch_h2(agout, t, 0)
                        fetch_h3(agout, t - 1, B)

                # ---------- output projection every YG steps ----------
                if l3_active and t3 % YG == YG - 1:
                    grp_i = t3 // YG
                    grp = h3_group[grp_i]  # [128, YG, 8, B]
                    yp = yps.tile([F, YG * B], f32)
                    for k in range(8):
                        nc.tensor.matmul(yp[:], Wout[:, k], grp[:, :, k],
                                         start=(k == 0), stop=(k == 7))
                    ysb = sbt.tile([F, YG * B], f32, tag="ysb")
                    nc.scalar.activation(ysb[:], yp[:], AF.Tanh, bias=bout[:])
                    nc.sync.dma_start(p_Y[:, grp_i * YG * B:(grp_i + 1) * YG * B], ysb[:])
                    h3_group.pop(grp_i - 1, None)
                    Hts[3] = {k_: v_ for k_, v_ in Hts[3].items() if k_ >= t3}

    nc.compile()
    return nc


_CACHED = {}


def _get_nc(t_steps=T):
    if t_steps not in _CACHED:
        _CACHED[t_steps] = build_nc(t_steps)
    return _CACHED[t_steps]


def make_in_maps(X, weights):
    return [_prep_core_inputs(r, X, weights) for r in range(R)]


def _weights_tuple(kw):
    return tuple(
        np.asarray(kw[k], np.float32)
        for k in ("w_ih1", "w_hh1", "b_ih1", "b_hh1", "w_ih2", "w_hh2", "b_ih2",
                  "b_hh2", "w_ih3", "w_hh3", "b_ih3", "b_hh3", "w_out", "b_out")
    )


def assemble_output(Y, t_steps=T):
    """[F, t*B] -> [B, t, F]"""
    return np.ascontiguousarray(Y.reshape(F, t_steps, B).transpose(2, 1, 0))


def kernel(X, **kw):
    from concourse.bass_utils import run_bass_kernel_spmd

    nc = _get_nc(T)
    in_maps = make_in_maps(np.asarray(X, np.float32), _weights_tuple(kw))
    res = run_bass_kernel_spmd(nc, in_maps, core_ids=list(range(R)))
    return assemble_output(res.results[0]["Y"])

